# revision 1
# baseline (speedup 1.0000x reference)
"""Trainium2 Bass kernel for gnn_message_passing (nn_BuildK_25005299597348).

Reference computation:
    UU = input1.reshape(32, N).T              # [N, 32] pixel features
    nbr = UU[input2]                          # [J, 48, 32] neighbor gather
    msd = mean((UU[:J, None, :] - nbr)**2, -1)
    W = softmax(-sqrt(msd + 1e-9), axis=1)    # [J, 48]

Strategy (8 NeuronCores, data-parallel over query rows):
  - Host: reshape input1 to [32, N]; build per-core index blocks idxq[J/8, 49]
    where column 0 is the query's own row index and 1..48 are its neighbors
    (so query rows are fetched by the same indirect gather - no per-core
    dynamic addressing, pure SPMD).
  - Device phase 1: transpose [32, N] -> row-major table uu[N, 32] in DRAM
    via PE transposes (128x128 tiles).
  - Device phase 2: per 512-query supertile, one indirect DMA gathers
    128x(4*49) rows of 32 f32; DVE/ACT compute diff, square, per-neighbor
    reduction, sqrt, and a fused softmax over the 48 neighbors.
"""

import sys

for _p in ("/opt/trn_rl_repo", "/root/.axon_site/_ro/trn_rl_repo"):
    if _p not in sys.path:
        sys.path.append(_p)

import numpy as np

import concourse.bass as bass
import concourse.bacc as bacc
import concourse.mybir as mybir
import concourse.tile as tile

F32 = mybir.dt.float32
I32 = mybir.dt.int32

N = 147456          # pixels (384*384)
A = 32              # features
K = 48              # neighbors
NCORES = 8
JC = N // NCORES    # queries per core (18432)
P = 128             # partitions
EPS = 1e-9


def build_kernel(n=N, a=A, k=K, jc=JC, t_blocks=4):
    """Build the SPMD Bass program. Returns nc."""
    kk = k + 1                      # query row + k neighbors
    nq = n // 4                     # columns per feature-quarter
    m0_cnt = n // (4 * P)           # 128-col blocks per quarter
    sup = jc // (P * t_blocks)      # supertiles per core
    assert n % (4 * P) == 0 and jc % (P * t_blocks) == 0

    nc = bacc.Bacc(None, target_bir_lowering=False)
    # register an SBUF constant for the sqrt bias (same pattern Bass.__init__
    # uses for 0.0/1.0)
    eps_t = nc.alloc_sbuf_tensor("const-eps", [P, 1], F32)
    nc.gpsimd.memset(eps_t.ap(), EPS)
    nc.const_aps.aps[(F32, EPS)] = eps_t.ap()
    nc.all_engine_barrier()

    feat = nc.declare_dram_parameter("feat", [a, n], F32, isOutput=False)
    ident_in = nc.declare_dram_parameter("ident", [P, P], F32, isOutput=False)
    idxq = nc.declare_dram_parameter("idxq", [jc, kk], I32, isOutput=False)
    out = nc.declare_dram_parameter("out", [jc, k], F32, isOutput=True)
    uu = nc.dram_tensor("uu", [n, a], F32)

    with tile.TileContext(nc) as tc:
        # ---------------- phase 1: feat [a, n] -> uu [n, a] ----------------
        with (
            tc.tile_pool(name="ph1c", bufs=4) as pc,
            tc.tile_pool(name="ph1s", bufs=4) as psg,
            tc.tile_pool(name="ph1p", bufs=4, space="PSUM") as pp,
            tc.tile_pool(name="ph1i", bufs=1) as pid,
        ):
            ident = pid.tile([P, P], F32)
            nc.sync.dma_start(out=ident[:], in_=ident_in[:])
            for m0 in range(m0_cnt):
                # cin[q*32+f, j] = feat[f, q*nq + m0*128 + j]
                cin = pc.tile([P, P], F32)
                src = bass.AP(
                    feat[:].tensor, m0 * P, [[nq, 4], [n, a], [1, P]]
                )
                nc.sync.dma_start(out=cin[:], in_=src)
                ps = pp.tile([P, P], F32)
                nc.tensor.transpose(out=ps[:], in_=cin[:], identity=ident[:])
                st = psg.tile([P, P], F32)
                nc.scalar.copy(out=st[:], in_=ps[:])
                # st[j, q*32+f] -> uu row (q*nq + m0*128 + j), feature f
                dst = bass.AP(
                    uu[:].tensor,
                    m0 * P * a,
                    [[a, P], [nq * a, 4], [1, a]],
                )
                nc.sync.dma_start(out=dst, in_=st[:])

        # ---------------- phase 2: gather + msd + softmax ----------------
        tkk = t_blocks * kk
        tk = t_blocks * k
        idxq_v = idxq[:].rearrange("(s t p) k -> s p t k", t=t_blocks, p=P)
        out_v = out[:].rearrange("(s t p) k -> s p t k", t=t_blocks, p=P)
        with (
            tc.tile_pool(name="pg", bufs=2) as pg,
            tc.tile_pool(name="pd", bufs=2) as pd,
            tc.tile_pool(name="psq", bufs=2) as psq,
            tc.tile_pool(name="pix", bufs=2) as pix,
            tc.tile_pool(name="psm", bufs=2) as psm,
            tc.tile_pool(name="pty", bufs=2) as pty,
        ):
            for s in range(sup):
                ix = pix.tile([P, tkk], I32)
                nc.sync.dma_start(
                    out=ix[:].rearrange("p (t k) -> p t k", k=kk), in_=idxq_v[s]
                )
                g = pg.tile([P, tkk * a], F32)
                # HW indirect DMA = one offset per partition, one contiguous
                # 128B descriptor per partition. Gather slot (t,k) for all 128
                # partition-queries per instruction.
                for m in range(tkk):
                    nc.gpsimd.indirect_dma_start(
                        out=g[:, m * a:(m + 1) * a],
                        out_offset=None,
                        in_=uu[:],
                        in_offset=bass.IndirectOffsetOnAxis(
                            ap=ix[:, m:m + 1], axis=0
                        ),
                    )
                g4 = g[:].rearrange("p (t kk f) -> p t kk f", t=t_blocks, kk=kk)
                diff = pd.tile([P, tk * a], F32)
                nc.vector.tensor_tensor(
                    out=diff[:].rearrange("p (t k f) -> p t k f", t=t_blocks, k=k),
                    in0=g4[:, :, 1:, :],
                    in1=g4[:, :, 0:1, :].to_broadcast([P, t_blocks, k, a]),
                    op=mybir.AluOpType.subtract,
                )
                sq = psq.tile([P, tk * a], F32)
                nc.scalar.square(out=sq[:], in_=diff[:])
                ss = psm.tile([P, tk], F32)
                nc.vector.tensor_reduce(
                    out=ss[:],
                    in_=sq[:].rearrange("p (m f) -> p m f", f=a),
                    axis=mybir.AxisListType.X,
                    op=mybir.AluOpType.add,
                )
                # sdist = sqrt(ss/a + eps); D = -sdist
                sd = psm.tile([P, tk], F32)
                nc.scalar.activation(
                    out=sd[:], in_=ss[:], func=mybir.ActivationFunctionType.Sqrt,
                    bias=EPS, scale=1.0 / a,
                )
                mn = pty.tile([P, t_blocks], F32)
                nc.vector.tensor_reduce(
                    out=mn[:],
                    in_=sd[:].rearrange("p (t k) -> p t k", k=k),
                    axis=mybir.AxisListType.X,
                    op=mybir.AluOpType.min,
                )
                sm = psm.tile([P, tk], F32)
                nc.vector.tensor_tensor(
                    out=sm[:].rearrange("p (t k) -> p t k", k=k),
                    in0=sd[:].rearrange("p (t k) -> p t k", k=k),
                    in1=mn[:].rearrange("p (t o) -> p t o", o=1).to_broadcast(
                        [P, t_blocks, k]
                    ),
                    op=mybir.AluOpType.subtract,
                )
                ex = psm.tile([P, tk], F32)
                nc.scalar.activation(
                    out=ex[:], in_=sm[:], func=mybir.ActivationFunctionType.Exp,
                    scale=-1.0,
                )
                se = pty.tile([P, t_blocks], F32)
                nc.vector.tensor_reduce(
                    out=se[:],
                    in_=ex[:].rearrange("p (t k) -> p t k", k=k),
                    axis=mybir.AxisListType.X,
                    op=mybir.AluOpType.add,
                )
                rc = pty.tile([P, t_blocks], F32)
                nc.vector.reciprocal(out=rc[:], in_=se[:])
                wt = psm.tile([P, tk], F32)
                nc.vector.tensor_tensor(
                    out=wt[:].rearrange("p (t k) -> p t k", k=k),
                    in0=ex[:].rearrange("p (t k) -> p t k", k=k),
                    in1=rc[:].rearrange("p (t o) -> p t o", o=1).to_broadcast(
                        [P, t_blocks, k]
                    ),
                    op=mybir.AluOpType.mult,
                )
                nc.sync.dma_start(
                    out=out_v[s], in_=wt[:].rearrange("p (t k) -> p t k", k=k)
                )
    return nc


_compiled = {}


def _run(input1, input2, trace=False, **trace_kwargs):
    from concourse.bass_utils import run_bass_kernel_spmd

    feat = np.ascontiguousarray(
        np.asarray(input1, dtype=np.float32).reshape(A, N)
    )
    idx = np.asarray(input2).astype(np.int32)           # [N, K]
    qi = np.arange(N, dtype=np.int32)[:, None]
    idxq = np.ascontiguousarray(np.concatenate([qi, idx], axis=1))  # [N, K+1]

    if "nc" not in _compiled:
        nc = build_kernel()
        nc.finalize()  # run the Bacc legalization passes (reg alloc, sync-wait split)
        _compiled["nc"] = nc
    nc = _compiled["nc"]

    ident = np.eye(P, dtype=np.float32)
    in_maps = [
        {"feat": feat, "idxq": idxq[c * JC:(c + 1) * JC], "ident": ident}
        for c in range(NCORES)
    ]
    res = run_bass_kernel_spmd(
        nc, in_maps, list(range(NCORES)), trace=trace, **trace_kwargs
    )
    out = np.concatenate(
        [res.results[c]["out"] for c in range(NCORES)], axis=0
    )
    return out, res


def kernel(input1: np.ndarray, input2: np.ndarray) -> np.ndarray:
    out, _ = _run(input1, input2)
    return out



# revision 16
# speedup vs baseline: 1.0665x; 1.0665x over previous
"""Trainium2 Bass kernel for gnn_message_passing (nn_BuildK_25005299597348).

Reference computation:
    UU = input1.reshape(32, N).T              # [N, 32] pixel features
    nbr = UU[input2]                          # [J, 48, 32] neighbor gather
    msd = mean((UU[:J, None, :] - nbr)**2, -1)
    W = softmax(-sqrt(msd + 1e-9), axis=1)    # [J, 48]

Strategy (8 NeuronCores, data-parallel over query rows):
  - Host: build the gather table uu16 = UU.astype(fp16) [N, 32] (64 B rows,
    replicated to every core), per-core neighbor indices idxq [J/8, 48] i32,
    and per-core query features qf = uu16[core slice] (loaded by regular DMA
    since query rows are contiguous).
  - Device, per 512-query supertile: batched indirect DMAs (48 gather slots
    x 128 partitions = 6144 rows each; 12288 m2s+s2m descriptors, safely
    under the 16384-slot SWDGE descriptor ring) fetch neighbor rows.  The
    baseline's per-instruction SWDGE fixed cost (7056 instrs x ~1.13 us of
    Pool time) was the original bottleneck.
  - Compute (fp16 to enable DVE 2x_1p): DVE subtract; square split between
    DVE (tensor_tensor mult) and ACT (Square) to balance engines; the
    32-feature reduction as a tree of tensor_tensor adds (tensor_reduce has
    no fast mode and measured ~2x slower); ACT sqrt+exp; DVE softmax.
    exp(-sd) needs no max-subtraction: sd in [0, ~4] so exp in [0.018, 1].
"""

import sys

for _p in ("/opt/trn_rl_repo", "/root/.axon_site/_ro/trn_rl_repo"):
    if _p not in sys.path:
        sys.path.append(_p)

import numpy as np

import concourse.bass as bass
import concourse.bacc as bacc
import concourse.mybir as mybir
import concourse.tile as tile

F32 = mybir.dt.float32
F16 = mybir.dt.float16
I32 = mybir.dt.int32

N = 147456          # pixels (384*384)
A = 32              # features
K = 48              # neighbors
NCORES = 8
JC = N // NCORES    # queries per core (18432)
P = 128             # partitions
EPS = 1e-9

NQUEUES = 1         # SWDGE queues to spread gather instructions across
ACT_SQ_FRAC = 0.5   # fraction of the square done on ACT (rest on DVE)


def build_kernel(n=N, a=A, k=K, jc=JC, t_blocks=4, debug_dump=False):
    """Build the SPMD Bass program. Returns nc."""
    tk = t_blocks * k               # gather slots per partition per supertile
    sup = jc // (P * t_blocks)      # supertiles per core
    assert jc % (P * t_blocks) == 0

    nc = bacc.Bacc(None, target_bir_lowering=False, num_swdge_queues=NQUEUES)
    # register an SBUF constant for the sqrt bias (same pattern Bass.__init__
    # uses for 0.0/1.0)
    eps_t = nc.alloc_sbuf_tensor("const-eps", [P, 1], F32)
    nc.gpsimd.memset(eps_t.ap(), EPS)
    nc.const_aps.aps[(F32, EPS)] = eps_t.ap()
    nc.all_engine_barrier()

    uu16 = nc.declare_dram_parameter("uu16", [n, a], F16, isOutput=False)
    qf = nc.declare_dram_parameter("qf", [jc, a], F16, isOutput=False)
    idxq = nc.declare_dram_parameter("idxq", [jc, k], I32, isOutput=False)
    out = nc.declare_dram_parameter("out", [jc, k], F32, isOutput=True)
    if debug_dump:
        gdump = nc.declare_dram_parameter(
            "gdump", [sup, P, tk * a], F16, isOutput=True
        )
        ixdump = nc.declare_dram_parameter(
            "ixdump", [sup, P, tk], I32, isOutput=True
        )

    ix_v = idxq[:].rearrange("(s t p) k -> s p t k", t=t_blocks, p=P)
    qf_v = qf[:].rearrange("(s t p) f -> s p t f", t=t_blocks, p=P)
    out_v = out[:].rearrange("(s t p) k -> s p t k", t=t_blocks, p=P)

    n_act = int(tk * ACT_SQ_FRAC) * a   # columns squared on ACT

    with tile.TileContext(nc) as tc:
        with (
            tc.tile_pool(name="pg", bufs=3) as pg,
            tc.tile_pool(name="pd", bufs=2) as pd,
            tc.tile_pool(name="psq", bufs=2) as psq,
            tc.tile_pool(name="ph", bufs=2) as ph,
            tc.tile_pool(name="pix", bufs=2) as pix,
            tc.tile_pool(name="pqf", bufs=3) as pqf,
            tc.tile_pool(name="psm", bufs=2) as psm,
            tc.tile_pool(name="pty", bufs=2) as pty,
        ):
            for s in range(sup):
                ix = pix.tile([P, tk], I32)
                nc.sync.dma_start(
                    out=ix[:].rearrange("p (t k) -> p t k", k=k), in_=ix_v[s]
                )
                qt = pqf.tile([P, t_blocks * a], F16)
                nc.sync.dma_start(
                    out=qt[:].rearrange("p (t f) -> p t f", f=a), in_=qf_v[s]
                )
                g = pg.tile([P, tk * a], F16)
                # HW indirect DMA semantics: ONE offset per partition per
                # instruction (extra offset columns are ignored and the
                # destination free-extent is read as a CONTIGUOUS run from
                # the first offset).  So: one instruction per (t, k) slot.
                for m in range(tk):
                    inst = nc.gpsimd.indirect_dma_start(
                        out=g[:, m * a:(m + 1) * a],
                        out_offset=None,
                        in_=uu16[:],
                        in_offset=bass.IndirectOffsetOnAxis(
                            ap=ix[:, m:m + 1], axis=0
                        ),
                    )
                    if NQUEUES > 1:
                        qn = m % NQUEUES
                        inst.ins.queue = f"qPoolDynamic{qn or ''}" 
                if debug_dump:
                    nc.sync.dma_start(out=gdump[s], in_=g[:])
                    nc.sync.dma_start(out=ixdump[s], in_=ix[:])
                g4 = g[:].rearrange("p (t k f) -> p t k f", t=t_blocks, k=k)
                q4 = qt[:].rearrange("p (t o f) -> p t o f", t=t_blocks, o=1)
                diff = pd.tile([P, tk * a], F16)
                nc.vector.tensor_tensor(
                    out=diff[:].rearrange(
                        "p (t k f) -> p t k f", t=t_blocks, k=k
                    ),
                    in0=g4,
                    in1=q4.to_broadcast([P, t_blocks, k, a]),
                    op=mybir.AluOpType.subtract,
                )
                sq = psq.tile([P, tk * a], F16)
                if n_act:
                    nc.scalar.square(out=sq[:, :n_act], in_=diff[:, :n_act])
                if n_act < tk * a:
                    nc.vector.tensor_tensor(
                        out=sq[:, n_act:],
                        in0=diff[:, n_act:],
                        in1=diff[:, n_act:],
                        op=mybir.AluOpType.mult,
                    )
                # 32 -> 1 reduction as a tree of fp16 tensor_tensor adds
                # (2x_1p-capable), final 2 -> 1 level via tensor_reduce to f32.
                sq3 = sq[:].rearrange("p (m f) -> p m f", f=a)
                h1 = ph.tile([P, tk * 16], F16, tag="h1")
                h1v = h1[:].rearrange("p (m f) -> p m f", f=16)
                nc.vector.tensor_tensor(
                    out=h1v, in0=sq3[:, :, 0:16], in1=sq3[:, :, 16:32],
                    op=mybir.AluOpType.add,
                )
                h2 = ph.tile([P, tk * 8], F16, tag="h2")
                h2v = h2[:].rearrange("p (m f) -> p m f", f=8)
                nc.vector.tensor_tensor(
                    out=h2v, in0=h1v[:, :, 0:8], in1=h1v[:, :, 8:16],
                    op=mybir.AluOpType.add,
                )
                h3 = ph.tile([P, tk * 4], F16, tag="h3")
                h3v = h3[:].rearrange("p (m f) -> p m f", f=4)
                nc.vector.tensor_tensor(
                    out=h3v, in0=h2v[:, :, 0:4], in1=h2v[:, :, 4:8],
                    op=mybir.AluOpType.add,
                )
                h4 = ph.tile([P, tk * 2], F16, tag="h4")
                h4v = h4[:].rearrange("p (m f) -> p m f", f=2)
                nc.vector.tensor_tensor(
                    out=h4v, in0=h3v[:, :, 0:2], in1=h3v[:, :, 2:4],
                    op=mybir.AluOpType.add,
                )
                ss = psm.tile([P, tk], F32)
                nc.vector.tensor_reduce(
                    out=ss[:], in_=h4v,
                    axis=mybir.AxisListType.X, op=mybir.AluOpType.add,
                )
                # sd = sqrt(ss/a + eps); D = -sd
                sd = psm.tile([P, tk], F32)
                nc.scalar.activation(
                    out=sd[:], in_=ss[:],
                    func=mybir.ActivationFunctionType.Sqrt,
                    bias=EPS, scale=1.0 / a,
                )
                ex = psm.tile([P, tk], F32)
                nc.scalar.activation(
                    out=ex[:], in_=sd[:],
                    func=mybir.ActivationFunctionType.Exp,
                    scale=-1.0,
                )
                se = pty.tile([P, t_blocks], F32)
                nc.vector.tensor_reduce(
                    out=se[:],
                    in_=ex[:].rearrange("p (t k) -> p t k", k=k),
                    axis=mybir.AxisListType.X,
                    op=mybir.AluOpType.add,
                )
                rc = pty.tile([P, t_blocks], F32)
                nc.vector.reciprocal(out=rc[:], in_=se[:])
                wt = psm.tile([P, tk], F32)
                nc.vector.tensor_tensor(
                    out=wt[:].rearrange("p (t k) -> p t k", k=k),
                    in0=ex[:].rearrange("p (t k) -> p t k", k=k),
                    in1=rc[:].rearrange("p (t o) -> p t o", o=1).to_broadcast(
                        [P, t_blocks, k]
                    ),
                    op=mybir.AluOpType.mult,
                )
                nc.sync.dma_start(
                    out=out_v[s], in_=wt[:].rearrange("p (t k) -> p t k", k=k)
                )
    return nc


_compiled = {}


def _prep_inputs(input1, input2):
    feat = np.asarray(input1, dtype=np.float32).reshape(A, N)
    uu16 = np.ascontiguousarray(feat.T.astype(np.float16))  # [N, 32]
    idx = np.ascontiguousarray(np.asarray(input2).astype(np.int32))  # [N, K]
    return uu16, idx


def _run(input1, input2, trace=False, **trace_kwargs):
    from concourse.bass_utils import run_bass_kernel_spmd

    uu16, idx = _prep_inputs(input1, input2)

    if "nc" not in _compiled:
        nc = build_kernel()
        nc.finalize()  # run the Bacc legalization passes
        _compiled["nc"] = nc
    nc = _compiled["nc"]

    in_maps = [
        {
            "uu16": uu16,
            "qf": uu16[c * JC:(c + 1) * JC],
            "idxq": idx[c * JC:(c + 1) * JC],
        }
        for c in range(NCORES)
    ]
    res = run_bass_kernel_spmd(
        nc, in_maps, list(range(NCORES)), trace=trace, **trace_kwargs
    )
    out = np.concatenate(
        [res.results[c]["out"] for c in range(NCORES)], axis=0
    )
    return out, res


def kernel(input1: np.ndarray, input2: np.ndarray) -> np.ndarray:
    out, _ = _run(input1, input2)
    return out


# revision 17
# speedup vs baseline: 1.0685x; 1.0018x over previous
"""Trainium2 Bass kernel for gnn_message_passing (nn_BuildK_25005299597348).

Reference computation:
    UU = input1.reshape(32, N).T              # [N, 32] pixel features
    nbr = UU[input2]                          # [J, 48, 32] neighbor gather
    msd = mean((UU[:J, None, :] - nbr)**2, -1)
    W = softmax(-sqrt(msd + 1e-9), axis=1)    # [J, 48]

Strategy (8 NeuronCores, data-parallel over query rows):
  - Host: build the gather table uu16 = UU.astype(fp16) [N, 32] (64 B rows,
    replicated to every core), per-core neighbor indices idxq [J/8, 48] i32,
    and per-core query features qf = uu16[core slice] (loaded by regular DMA
    since query rows are contiguous).
  - Device, per 512-query supertile: 192 indirect DMAs fetch the neighbor
    rows.  HW constraint (verified empirically): each indirect DMA takes
    exactly ONE offset per destination partition, read from partition p of
    the offset AP, and fetches the destination free-extent as a CONTIGUOUS
    run from that row.  So 48 slots x 4 t-blocks x 36 supertiles = 6912
    instructions per core is the minimum for this gather, at ~1.1 us of
    Pool-engine (Q7 SWDGE) time each.
  - Compute (fp16 to enable DVE 2x_1p): DVE subtract; square split between
    DVE (tensor_tensor mult) and ACT (Square) to balance engines; the
    32-feature reduction as a tree of tensor_tensor adds (tensor_reduce has
    no fast mode and measured ~2x slower); ACT sqrt+exp; DVE softmax.
    exp(-sd) needs no max-subtraction: sd in [0, ~4] so exp in [0.018, 1].
"""

import sys

for _p in ("/opt/trn_rl_repo", "/root/.axon_site/_ro/trn_rl_repo"):
    if _p not in sys.path:
        sys.path.append(_p)

import numpy as np

import concourse.bass as bass
import concourse.bacc as bacc
import concourse.mybir as mybir
import concourse.tile as tile

F32 = mybir.dt.float32
F16 = mybir.dt.float16
I32 = mybir.dt.int32

N = 147456          # pixels (384*384)
A = 32              # features
K = 48              # neighbors
NCORES = 8
JC = N // NCORES    # queries per core (18432)
P = 128             # partitions
EPS = 1e-9

NQUEUES = 1         # SWDGE queues to spread gather instructions across
ACT_SQ_FRAC = 0.5   # fraction of the square done on ACT (rest on DVE)


def build_kernel(n=N, a=A, k=K, jc=JC, t_blocks=4, debug_dump=False):
    """Build the SPMD Bass program. Returns nc."""
    tk = t_blocks * k               # gather slots per partition per supertile
    sup = jc // (P * t_blocks)      # supertiles per core
    assert jc % (P * t_blocks) == 0

    nc = bacc.Bacc(None, target_bir_lowering=False, num_swdge_queues=NQUEUES)
    # register an SBUF constant for the sqrt bias (same pattern Bass.__init__
    # uses for 0.0/1.0)
    eps_t = nc.alloc_sbuf_tensor("const-eps", [P, 1], F32)
    nc.gpsimd.memset(eps_t.ap(), EPS)
    nc.const_aps.aps[(F32, EPS)] = eps_t.ap()
    nc.all_engine_barrier()

    uu16 = nc.declare_dram_parameter("uu16", [n, a], F16, isOutput=False)
    qf = nc.declare_dram_parameter("qf", [jc, a], F16, isOutput=False)
    idxq = nc.declare_dram_parameter("idxq", [jc, k], I32, isOutput=False)
    out = nc.declare_dram_parameter("out", [jc, k], F32, isOutput=True)
    if debug_dump:
        gdump = nc.declare_dram_parameter(
            "gdump", [sup, P, tk * a], F16, isOutput=True
        )
        ixdump = nc.declare_dram_parameter(
            "ixdump", [sup, P, tk], I32, isOutput=True
        )

    ix_v = idxq[:].rearrange("(s t p) k -> s p t k", t=t_blocks, p=P)
    qf_v = qf[:].rearrange("(s t p) f -> s p t f", t=t_blocks, p=P)
    out_v = out[:].rearrange("(s t p) k -> s p t k", t=t_blocks, p=P)

    n_act = int(tk * ACT_SQ_FRAC) * a   # columns squared on ACT

    with tile.TileContext(nc) as tc:
        with (
            tc.tile_pool(name="pg", bufs=3) as pg,
            tc.tile_pool(name="pd", bufs=2) as pd,
            tc.tile_pool(name="psq", bufs=2) as psq,
            tc.tile_pool(name="ph", bufs=2) as ph,
            tc.tile_pool(name="pix", bufs=2) as pix,
            tc.tile_pool(name="pqf", bufs=3) as pqf,
            tc.tile_pool(name="psm", bufs=2) as psm,
            tc.tile_pool(name="pty", bufs=2) as pty,
        ):
            for s in range(sup):
                ix = pix.tile([P, tk], I32)
                nc.sync.dma_start(
                    out=ix[:].rearrange("p (t k) -> p t k", k=k), in_=ix_v[s]
                )
                qt = pqf.tile([P, t_blocks * a], F16)
                nc.sync.dma_start(
                    out=qt[:].rearrange("p (t f) -> p t f", f=a), in_=qf_v[s]
                )
                g = pg.tile([P, tk * a], F16)
                # HW indirect DMA semantics: ONE offset per partition per
                # instruction (extra offset columns are ignored and the
                # destination free-extent is read as a CONTIGUOUS run from
                # the first offset).  So: one instruction per (t, k) slot.
                for m in range(tk):
                    inst = nc.gpsimd.indirect_dma_start(
                        out=g[:, m * a:(m + 1) * a],
                        out_offset=None,
                        in_=uu16[:],
                        in_offset=bass.IndirectOffsetOnAxis(
                            ap=ix[:, m:m + 1], axis=0
                        ),
                    )
                    if NQUEUES > 1:
                        qn = m % NQUEUES
                        inst.ins.queue = f"qPoolDynamic{qn or ''}"
                if debug_dump:
                    nc.sync.dma_start(out=gdump[s], in_=g[:])
                    nc.sync.dma_start(out=ixdump[s], in_=ix[:])
                g4 = g[:].rearrange("p (t k f) -> p t k f", t=t_blocks, k=k)
                q4 = qt[:].rearrange("p (t o f) -> p t o f", t=t_blocks, o=1)
                diff = pd.tile([P, tk * a], F16)
                nc.vector.tensor_tensor(
                    out=diff[:].rearrange(
                        "p (t k f) -> p t k f", t=t_blocks, k=k
                    ),
                    in0=g4,
                    in1=q4.to_broadcast([P, t_blocks, k, a]),
                    op=mybir.AluOpType.subtract,
                )
                sq = psq.tile([P, tk * a], F16)
                if n_act:
                    nc.scalar.square(out=sq[:, :n_act], in_=diff[:, :n_act])
                if n_act < tk * a:
                    nc.vector.tensor_tensor(
                        out=sq[:, n_act:],
                        in0=diff[:, n_act:],
                        in1=diff[:, n_act:],
                        op=mybir.AluOpType.mult,
                    )
                # 32 -> 1 reduction as a tree of fp16 tensor_tensor adds
                # (2x_1p-capable), final 2 -> 1 level via tensor_reduce to f32.
                sq3 = sq[:].rearrange("p (m f) -> p m f", f=a)
                h1 = ph.tile([P, tk * 16], F16, tag="h1")
                h1v = h1[:].rearrange("p (m f) -> p m f", f=16)
                nc.vector.tensor_tensor(
                    out=h1v, in0=sq3[:, :, 0:16], in1=sq3[:, :, 16:32],
                    op=mybir.AluOpType.add,
                )
                h2 = ph.tile([P, tk * 8], F16, tag="h2")
                h2v = h2[:].rearrange("p (m f) -> p m f", f=8)
                nc.vector.tensor_tensor(
                    out=h2v, in0=h1v[:, :, 0:8], in1=h1v[:, :, 8:16],
                    op=mybir.AluOpType.add,
                )
                h3 = ph.tile([P, tk * 4], F16, tag="h3")
                h3v = h3[:].rearrange("p (m f) -> p m f", f=4)
                nc.vector.tensor_tensor(
                    out=h3v, in0=h2v[:, :, 0:4], in1=h2v[:, :, 4:8],
                    op=mybir.AluOpType.add,
                )
                h4 = ph.tile([P, tk * 2], F16, tag="h4")
                h4v = h4[:].rearrange("p (m f) -> p m f", f=2)
                nc.vector.tensor_tensor(
                    out=h4v, in0=h3v[:, :, 0:2], in1=h3v[:, :, 2:4],
                    op=mybir.AluOpType.add,
                )
                ss = psm.tile([P, tk], F32)
                nc.vector.tensor_reduce(
                    out=ss[:], in_=h4v,
                    axis=mybir.AxisListType.X, op=mybir.AluOpType.add,
                )
                # sd = sqrt(ss/a + eps); D = -sd
                sd = psm.tile([P, tk], F32)
                nc.scalar.activation(
                    out=sd[:], in_=ss[:],
                    func=mybir.ActivationFunctionType.Sqrt,
                    bias=EPS, scale=1.0 / a,
                )
                ex = psm.tile([P, tk], F32)
                nc.scalar.activation(
                    out=ex[:], in_=sd[:],
                    func=mybir.ActivationFunctionType.Exp,
                    scale=-1.0,
                )
                se = pty.tile([P, t_blocks], F32)
                nc.vector.tensor_reduce(
                    out=se[:],
                    in_=ex[:].rearrange("p (t k) -> p t k", k=k),
                    axis=mybir.AxisListType.X,
                    op=mybir.AluOpType.add,
                )
                rc = pty.tile([P, t_blocks], F32)
                nc.vector.reciprocal(out=rc[:], in_=se[:])
                wt = psm.tile([P, tk], F32)
                nc.vector.tensor_tensor(
                    out=wt[:].rearrange("p (t k) -> p t k", k=k),
                    in0=ex[:].rearrange("p (t k) -> p t k", k=k),
                    in1=rc[:].rearrange("p (t o) -> p t o", o=1).to_broadcast(
                        [P, t_blocks, k]
                    ),
                    op=mybir.AluOpType.mult,
                )
                nc.sync.dma_start(
                    out=out_v[s], in_=wt[:].rearrange("p (t k) -> p t k", k=k)
                )
    return nc


_compiled = {}


def _prep_inputs(input1, input2):
    feat = np.asarray(input1, dtype=np.float32).reshape(A, N)
    uu16 = np.ascontiguousarray(feat.T.astype(np.float16))  # [N, 32]
    idx = np.ascontiguousarray(np.asarray(input2).astype(np.int32))  # [N, K]
    return uu16, idx


def _run(input1, input2, trace=False, **trace_kwargs):
    from concourse.bass_utils import run_bass_kernel_spmd

    uu16, idx = _prep_inputs(input1, input2)

    if "nc" not in _compiled:
        nc = build_kernel()
        nc.finalize()  # run the Bacc legalization passes
        _compiled["nc"] = nc
    nc = _compiled["nc"]

    in_maps = [
        {
            "uu16": uu16,
            "qf": uu16[c * JC:(c + 1) * JC],
            "idxq": idx[c * JC:(c + 1) * JC],
        }
        for c in range(NCORES)
    ]
    res = run_bass_kernel_spmd(
        nc, in_maps, list(range(NCORES)), trace=trace, **trace_kwargs
    )
    out = np.concatenate(
        [res.results[c]["out"] for c in range(NCORES)], axis=0
    )
    return out, res


def kernel(input1: np.ndarray, input2: np.ndarray) -> np.ndarray:
    out, _ = _run(input1, input2)
    return out
